# revision 7
# baseline (speedup 1.0000x reference)
"""GNN message-passing cell (3-step, 6 SpMMs) on 8 Trainium2 NeuronCores.

Strategy: 1D dest-node sharding. Each core owns 6272 rows (49 tiles of 128).
Per SpMM, edges are grouped by (dest core, source arrival-phase, dest tile);
neighbor features are fetched with dma_gather from an HBM-replicated state
table (built via chunked AllGather), scaled+segment-summed with a one-hot
matmul on the PE, accumulated in an SBUF accumulator, then LayerNorm+GELU.

v2: bf16 tables/streams, duplicate-spmm dedup (same adjacency+state computed
once, reused via an SBUF save buffer), per-tile-group gathers with trailing
negative indices so padded descriptors are skipped by SWDGE, fused one-hot
build (single tensor_scalar is_equal*val per chunk), affine/exchange
interleave, and LayerNorm fused per-tile into the last stream.
"""
import os
import sys

sys.path.insert(0, "/opt/trn_rl_repo")

import numpy as np

# ---------------- problem constants (hardcoded; must match reference) -------
N_STEP = 3
N_NODES = 50000
N_ADJ = 6
NNZ = 800000
DIN = 256
D = 128
LN_EPS = 1e-5

NCORES = 8
P = 128
TPC = 49                  # dest tiles per core
RPC = TPC * P             # 6272 rows per core
NPAD = NCORES * RPC       # 50176 padded rows
C0_T, C1_T = 25, 24       # shard-chunk split in tiles (for 2-chunk AllGather)
C0, C1 = C0_T * P, C1_T * P          # 3200 / 3072 rows per core per chunk
T0, T1 = NCORES * C0, NCORES * C1    # table chunk sizes: 25600 / 24576
GMAX = int(os.environ.get("KERNEL_GMAX", "8"))  # max chunks per dma_gather
                          # (8*128=1024 idxs; larger overflows the SWDGE
                          # descriptor carveout and wedges the device)
NSWQ = 4                  # SWDGE queues to rotate gathers over

DT_BF16 = os.environ.get("KERNEL_BF16", "1") == "1"
NEGPAD = os.environ.get("KERNEL_NEGPAD", "1") == "1"
DBG_STEPS = int(os.environ.get("KERNEL_STEPS", str(N_STEP)))  # debug bisection

LAST_RESULTS = {}         # test.py introspection (exec_time etc.)


# ---------------- host-side edge preprocessing ------------------------------
def _prep_spmm(rows, cols, vals):
    """Partition/sort/pad one adjacency's edges.

    Returns per-phase dicts with padded streams (tidx int16, dloc, val) laid
    out per core, plus the shared per-tile chunk counts K[p][t]. Padded slots
    get tidx=-1 (trailing within each tile group) so SWDGE skips their
    descriptors; dloc/val stay 0 so their one-hot column is zero.
    """
    rows = rows.astype(np.int64)
    cols = cols.astype(np.int64)
    dc = rows // RPC
    lr = rows % RPC
    t = lr // P
    dloc = lr % P
    cs = cols // RPC
    ls = cols % RPC
    ph = (ls >= C0).astype(np.int64)
    tidx = np.where(ph == 0, cs * C0 + ls, cs * C1 + (ls - C0))

    key = (dc * 2 + ph) * TPC + t
    # secondary sort by source index inside each group: gathered rows then
    # arrive mostly ascending -> HBM page locality for the random reads
    order = np.argsort(key * (1 << 16) + tidx, kind="stable")
    counts = np.bincount(key, minlength=NCORES * 2 * TPC).reshape(NCORES, 2, TPC)
    K = np.ceil(counts / P).astype(np.int64).max(axis=0)  # [2, TPC]
    K = np.maximum(K, 1)

    src_off = np.zeros(NCORES * 2 * TPC + 1, np.int64)
    np.cumsum(counts.reshape(-1), out=src_off[1:])
    # stream offsets per phase (tile groups packed in order)
    toff = [np.concatenate([[0], np.cumsum(K[p] * P)]) for p in range(2)]
    L = [int(toff[p][-1]) for p in range(2)]

    tidx_s = tidx[order]
    dloc_s = dloc[order]
    val_s = vals[order]

    pad_idx = -1 if NEGPAD else 0
    out = []
    for p in range(2):
        ti = np.full((NCORES, L[p]), pad_idx, np.int16)
        dl = np.zeros((NCORES, L[p]), np.float32)
        vl = np.zeros((NCORES, L[p]), np.float32)
        for c in range(NCORES):
            for tt in range(TPC):
                g = (c * 2 + p) * TPC + tt
                a, b = src_off[g], src_off[g + 1]
                n = b - a
                if n == 0:
                    continue
                o = toff[p][tt]
                ti[c, o:o + n] = tidx_s[a:b]
                dl[c, o:o + n] = dloc_s[a:b]
                vl[c, o:o + n] = val_s[a:b]
        out.append(dict(tidx=ti, dloc=dl, val=vl, L=L[p], K=K[p]))
    return out


def _groups(K):
    """Chunk-range [ga, gb) per dest tile from per-tile chunk counts."""
    g0 = np.concatenate([[0], np.cumsum(K)]).astype(np.int64)
    return [(t, int(g0[t]), int(g0[t + 1])) for t in range(TPC)]


def _wrap_idx(a):  # [L] int16 -> [128, L/16]
    return np.tile(a.reshape(-1, 16).T, (NCORES, 1)).astype(np.int16)


def _wrap_pe(a, np_dt):  # [L] -> [128, L/128] (edge e -> partition e%128)
    return np.ascontiguousarray(a.reshape(-1, P).T.astype(np_dt))


# ---------------- bass program ----------------------------------------------
def _build(meta):
    import concourse.bacc as bacc
    import concourse.mybir as mybir
    import concourse.tile as tile

    f32 = mybir.dt.float32
    i16 = mybir.dt.int16
    DT = mybir.dt.bfloat16 if DT_BF16 else f32
    Alu = mybir.AluOpType
    Act = mybir.ActivationFunctionType

    nc = bacc.Bacc("TRN2", target_bir_lowering=False, debug=False,
                   num_devices=NCORES, num_swdge_queues=NSWQ)

    xt_d = nc.dram_tensor("xt", [DIN, RPC], DT, kind="ExternalInput")
    w0_d = nc.dram_tensor("w0", [P, D], DT, kind="ExternalInput")
    w1_d = nc.dram_tensor("w1", [P, D], DT, kind="ExternalInput")
    brep_d = nc.dram_tensor("brep", [P, D], f32, kind="ExternalInput")
    grep_d = nc.dram_tensor("grep", [P, D], f32, kind="ExternalInput")
    berep_d = nc.dram_tensor("berep", [P, D], f32, kind="ExternalInput")
    iota_d = nc.dram_tensor("iotar", [P, D], DT, kind="ExternalInput")
    idx_d, dloc_d, val_d = {}, {}, {}
    for (m, p), L in meta["lengths"].items():
        idx_d[(m, p)] = nc.dram_tensor(f"idx_{m}_{p}", [P, L // 16], i16,
                                       kind="ExternalInput")
        dloc_d[(m, p)] = nc.dram_tensor(f"dloc_{m}_{p}", [P, L // P], f32,
                                        kind="ExternalInput")
        val_d[(m, p)] = nc.dram_tensor(f"val_{m}_{p}", [P, L // P], f32,
                                       kind="ExternalInput")
    out_d = nc.dram_tensor("out", [RPC, D], f32, kind="ExternalOutput")

    ts = lambda t: slice(t * D, (t + 1) * D)
    step_items = meta["steps"][:DBG_STEPS]
    dup_sources = meta["dup_sources"]
    KMAX = meta["kmax"]

    with tile.TileContext(nc) as tc:
        with (
            tc.tile_pool(name="const", bufs=1) as cp,
            tc.tile_pool(name="acc", bufs=1) as ap_,
            tc.tile_pool(name="xp", bufs=3) as xp,
            tc.tile_pool(name="gp", bufs=8) as gp,
            tc.tile_pool(name="op", bufs=8) as op_,
            tc.tile_pool(name="ip", bufs=2) as ip,
            tc.tile_pool(name="sp", bufs=4) as stp,
            tc.tile_pool(name="ps", bufs=8, space="PSUM") as pp,
            tc.tile_pool(name="dr", bufs=1, space="DRAM") as dp,
        ):
            w0_t = cp.tile([P, D], DT, name="w0t")
            nc.sync.dma_start(w0_t[:], w0_d[:])
            w1_t = cp.tile([P, D], DT, name="w1t")
            nc.sync.dma_start(w1_t[:], w1_d[:])
            brep_t = cp.tile([P, D], f32, name="brept")
            nc.sync.dma_start(brep_t[:], brep_d[:])
            grep_t = cp.tile([P, D], f32, name="grept")
            nc.sync.dma_start(grep_t[:], grep_d[:])
            berep_t = cp.tile([P, D], f32, name="berept")
            nc.sync.dma_start(berep_t[:], berep_d[:])
            iota_t = cp.tile([P, D], DT, name="iotat")
            nc.sync.dma_start(iota_t[:], iota_d[:])
            eps_t = cp.tile([P, 1], f32, name="epst")
            nc.vector.memset(eps_t[:], LN_EPS)

            accum = ap_.tile([P, TPC * D], f32, name="accum")
            saves = {sid: ap_.tile([P, TPC * D], f32, name=f"save{sid}")
                     for sid in dup_sources}

            # memset gather tiles once: SWDGE skips negative (padded) indices,
            # leaving stale SBUF bytes that feed the PE via a zero one-hot
            # column -- they must be finite (0 * Inf/NaN would poison PSUM).
            for _ in range(8):
                gz = gp.tile([P, KMAX, D], DT, tag="g")
                nc.vector.memset(gz[:], 0.0)

            tables = {}

            def exchange_chunk(s, ch):
                eng = nc.gpsimd if DT_BF16 else nc.sync
                if ch == 0:
                    agi0 = dp.tile([C0, D], DT, name=f"agi0_{s}", tag=f"agi0_{s}")
                    tab0 = dp.tile([T0, D], DT, name=f"tab0_{s}", tag=f"tab0_{s}",
                                   addr_space="Shared")
                    eng.dma_start(
                        agi0[:].rearrange("(t p) f -> p t f", p=P),
                        accum[:, :C0_T * D].rearrange("p (t f) -> p t f", f=D))
                    nc.gpsimd.collective_compute(
                        "AllGather", Alu.bypass,
                        replica_groups=[list(range(NCORES))],
                        ins=[agi0[:]], outs=[tab0[:]])
                    tables.setdefault(s, [None, None])[0] = tab0
                else:
                    agi1 = dp.tile([C1, D], DT, name=f"agi1_{s}", tag=f"agi1_{s}")
                    tab1 = dp.tile([T1, D], DT, name=f"tab1_{s}", tag=f"tab1_{s}",
                                   addr_space="Shared")
                    eng.dma_start(
                        agi1[:].rearrange("(t p) f -> p t f", p=P),
                        accum[:, C0_T * D:].rearrange("p (t f) -> p t f", f=D))
                    nc.gpsimd.collective_compute(
                        "AllGather", Alu.bypass,
                        replica_groups=[list(range(NCORES))],
                        ins=[agi1[:]], outs=[tab1[:]])
                    tables.setdefault(s, [None, None])[1] = tab1

            # ---------------- LayerNorm + GELU (fused per-tile) -------------
            def ln_tile(t, src):
                X = src[:, ts(t)]
                s1 = stp.tile([P, 1], f32, tag="s1")
                nc.vector.reduce_sum(out=s1[:], in_=X, axis=mybir.AxisListType.X)
                mean = stp.tile([P, 1], f32, tag="mean")
                nc.vector.tensor_scalar_mul(out=mean[:], in0=s1[:],
                                            scalar1=1.0 / D)
                xc = xp.tile([P, D], f32, tag="xc")
                nc.vector.tensor_scalar(out=xc[:], in0=X, scalar1=mean[:],
                                        scalar2=None, op0=Alu.subtract)
                sq = xp.tile([P, D], f32, tag="sq")
                nc.vector.tensor_tensor(out=sq[:], in0=xc[:], in1=xc[:],
                                        op=Alu.mult)
                v1 = stp.tile([P, 1], f32, tag="v1")
                nc.vector.reduce_sum(out=v1[:], in_=sq[:], axis=mybir.AxisListType.X)
                sd = stp.tile([P, 1], f32, tag="sd")
                nc.scalar.activation(out=sd[:], in_=v1[:], func=Act.Sqrt,
                                     bias=eps_t[:], scale=1.0 / D)
                rstd = stp.tile([P, 1], f32, tag="rstd")
                nc.vector.reciprocal(out=rstd[:], in_=sd[:])
                y = xp.tile([P, D], f32, tag="y")
                nc.vector.tensor_scalar(out=y[:], in0=xc[:], scalar1=rstd[:],
                                        scalar2=None, op0=Alu.mult)
                nc.vector.tensor_tensor(out=y[:], in0=y[:], in1=grep_t[:],
                                        op=Alu.mult)
                nc.vector.tensor_tensor(out=y[:], in0=y[:], in1=berep_t[:],
                                        op=Alu.add)
                yo = xp.tile([P, D], f32, tag="yo")
                nc.scalar.activation(out=yo[:], in_=y[:], func=Act.Gelu)
                nc.sync.dma_start(out_d[t * P:(t + 1) * P, :], yo[:])

            # ---------------- affine: h0 = x @ W + b (interleaved with ------
            # ---------------- the first AllGather's two chunks) -------------
            for t in range(TPC):
                xt0 = xp.tile([P, P], DT, tag="xt0")
                nc.sync.dma_start(xt0[:], xt_d[0:P, t * P:(t + 1) * P])
                xt1 = xp.tile([P, P], DT, tag="xt1")
                nc.sync.dma_start(xt1[:], xt_d[P:DIN, t * P:(t + 1) * P])
                ps = pp.tile([P, D], mybir.dt.float32, name="psa", tag="ps")
                nc.tensor.matmul(out=ps[:], lhsT=xt0[:], rhs=w0_t[:],
                                 start=True, stop=False)
                nc.tensor.matmul(out=ps[:], lhsT=xt1[:], rhs=w1_t[:],
                                 start=False, stop=True)
                nc.vector.tensor_tensor(out=accum[:, ts(t)], in0=ps[:],
                                        in1=brep_t[:], op=Alu.add)
                if t == C0_T - 1:
                    exchange_chunk(0, 0)
            exchange_chunk(0, 1)

            # ---------------- message-passing steps ----------------
            gq = [0]  # gather queue rotation counter

            def do_gather(tab, idx_t, ga, gb, qoff):
                """Gather chunks [ga, gb) of one tile group (split to <=GMAX)."""
                g_t = gp.tile([P, KMAX, D], DT, tag="g")
                o = ga
                while o < gb:
                    n = min(GMAX, gb - o)
                    nc.gpsimd.dma_gather(
                        g_t[:, o - ga:o - ga + n, :], tab[:],
                        idx_t[:, o * 8:(o + n) * 8],
                        n * P, n * P, D,
                        queue_num=gq[0] % NSWQ)
                    gq[0] += 1
                    o += n
                return g_t

            for i, items in enumerate(step_items):
                first = [True] * TPC
                last_spmm = max((j for j, it in enumerate(items)
                                 if it[0] == "spmm"), default=None)
                for j, it in enumerate(items):
                    if it[0] == "reuse":
                        sv = saves[it[1]]
                        for t in range(TPC):
                            if first[t]:
                                nc.vector.tensor_copy(out=accum[:, ts(t)],
                                                      in_=sv[:, ts(t)])
                                first[t] = False
                            else:
                                nc.vector.tensor_tensor(
                                    out=accum[:, ts(t)], in0=accum[:, ts(t)],
                                    in1=sv[:, ts(t)], op=Alu.add)
                        continue
                    _, m, sigma = it
                    is_src = m in dup_sources
                    first_sv = [True] * TPC
                    is_last = (j == last_spmm)
                    final_step = (i == len(step_items) - 1) and i == N_STEP - 1
                    for p in range(2):
                        L = meta["lengths"][(m, p)]
                        idx_t = ip.tile([P, L // 16], i16, tag="idx")
                        nc.sync.dma_start(idx_t[:], idx_d[(m, p)][:])
                        dloc_t = ip.tile([P, L // P], f32, tag="dloc")
                        nc.sync.dma_start(dloc_t[:], dloc_d[(m, p)][:])
                        val_t = ip.tile([P, L // P], f32, tag="val")
                        nc.sync.dma_start(val_t[:], val_d[(m, p)][:])
                        tab = tables[sigma][p]
                        for (t, ga, gb) in meta["groups"][(m, p)]:
                            g_t = do_gather(tab, idx_t, ga, gb, gq)
                            ps = pp.tile([P, D], mybir.dt.float32,
                                         name="psm", tag="ps")
                            for kk in range(ga, gb):
                                oh = op_.tile([P, D], DT, tag="oh")
                                nc.vector.tensor_scalar(
                                    out=oh[:], in0=iota_t[:],
                                    scalar1=dloc_t[:, kk:kk + 1],
                                    scalar2=val_t[:, kk:kk + 1],
                                    op0=Alu.is_equal, op1=Alu.mult)
                                nc.tensor.matmul(
                                    out=ps[:], lhsT=oh[:],
                                    rhs=g_t[:, kk - ga, :],
                                    start=(kk == ga), stop=(kk == gb - 1))
                            if first[t]:
                                nc.vector.tensor_copy(out=accum[:, ts(t)],
                                                      in_=ps[:])
                                first[t] = False
                            else:
                                nc.vector.tensor_tensor(
                                    out=accum[:, ts(t)],
                                    in0=accum[:, ts(t)], in1=ps[:],
                                    op=Alu.add)
                            if is_src:
                                if first_sv[t]:
                                    nc.vector.tensor_copy(
                                        out=saves[m][:, ts(t)], in_=ps[:])
                                    first_sv[t] = False
                                else:
                                    nc.vector.tensor_tensor(
                                        out=saves[m][:, ts(t)],
                                        in0=saves[m][:, ts(t)], in1=ps[:],
                                        op=Alu.add)
                            if is_last and p == 1:
                                # tile t is final for this step: kick off the
                                # next exchange chunk / fused LayerNorm early
                                if final_step:
                                    ln_tile(t, accum)
                                elif i < len(step_items) - 1:
                                    if t == C0_T - 1:
                                        exchange_chunk(i + 1, 0)
                                    elif t == TPC - 1:
                                        exchange_chunk(i + 1, 1)
                if last_spmm is None and i < len(step_items) - 1:
                    exchange_chunk(i + 1, 0)
                    exchange_chunk(i + 1, 1)

            if DBG_STEPS < N_STEP:  # debug: LN never fused, run it here
                for t in range(TPC):
                    ln_tile(t, accum)

    nc.compile()
    n_inst = sum(len(b.instructions) for f in nc.m.functions for b in f.blocks)
    print(f"[kernel] instructions: {n_inst}", flush=True)
    return nc


# ---------------- entry point ------------------------------------------------
def kernel(x, adj_rows, adj_cols, adj_vals, idxes_seq, idxes_res, W, b,
           gamma, beta):
    from concourse.bass_utils import run_bass_kernel_spmd

    import ml_dtypes
    np_DT = ml_dtypes.bfloat16 if DT_BF16 else np.float32

    x = np.asarray(x, np.float32)
    W = np.asarray(W, np.float32)
    b = np.asarray(b, np.float32)
    gamma = np.asarray(gamma, np.float32)
    beta = np.asarray(beta, np.float32)
    adj_rows = np.asarray(adj_rows)
    adj_cols = np.asarray(adj_cols)
    adj_vals = np.asarray(adj_vals, np.float32)
    idxes_seq = np.asarray(idxes_seq).astype(np.int64)
    idxes_res = np.asarray(idxes_res).astype(np.int64)

    # spmm list per step with residuals first (their tables exist already, so
    # their gathers overlap the in-flight AllGather of the fresh state).
    # Duplicate (adjacency, state) pairs are computed once and reused via an
    # SBUF save buffer.
    stream_of = {}           # (a, sigma) -> stream id
    stream_specs = []        # sid -> (a, sigma)
    step_items = []          # per step: ("spmm", sid, sigma) | ("reuse", sid)
    dup_sources = set()
    off = 0
    for i in range(N_STEP):
        raw = []
        for j in range(i):
            raw.append((int(idxes_res[off + j]), j))
        raw.append((int(idxes_seq[i]), i))
        off += i
        items = []
        for key in raw:
            if key in stream_of:
                dup_sources.add(stream_of[key])
                items.append(("reuse", stream_of[key]))
            else:
                sid = len(stream_specs)
                stream_of[key] = sid
                stream_specs.append(key)
                items.append(("spmm", sid, key[1]))
        step_items.append(items)

    # host prep per unique stream
    lengths, groups = {}, {}
    per_core_streams = {}     # (sid,p) -> dict arrays per core
    kmax = 1
    for sid, (a, sigma) in enumerate(stream_specs):
        phases = _prep_spmm(adj_rows[a], adj_cols[a], adj_vals[a])
        for p in range(2):
            ph = phases[p]
            lengths[(sid, p)] = ph["L"]
            groups[(sid, p)] = _groups(ph["K"])
            kmax = max(kmax, int(ph["K"].max()))
            per_core_streams[(sid, p)] = ph

    meta = dict(lengths=lengths, groups=groups, steps=step_items,
                dup_sources=dup_sources, kmax=kmax)
    nc = _build(meta)

    # per-core inputs
    xpad = np.zeros((NPAD, DIN), np.float32)
    xpad[:N_NODES] = x
    xt_full = np.ascontiguousarray(xpad.T)

    iota_rep = np.tile(np.arange(D, dtype=np.float32), (P, 1))
    in_maps = []
    for c in range(NCORES):
        im = dict(
            xt=np.ascontiguousarray(
                xt_full[:, c * RPC:(c + 1) * RPC]).astype(np_DT),
            w0=W[:P].astype(np_DT),
            w1=W[P:].astype(np_DT),
            brep=np.tile(b, (P, 1)).astype(np.float32),
            grep=np.tile(gamma, (P, 1)).astype(np.float32),
            berep=np.tile(beta, (P, 1)).astype(np.float32),
            iotar=iota_rep.astype(np_DT),
        )
        for (m, p), ph in per_core_streams.items():
            im[f"idx_{m}_{p}"] = _wrap_idx(ph["tidx"][c])
            im[f"dloc_{m}_{p}"] = _wrap_pe(ph["dloc"][c], np.float32)
            im[f"val_{m}_{p}"] = _wrap_pe(ph["val"][c], np.float32)
        in_maps.append(im)

    trace = os.environ.get("KERNEL_TRACE", "0") == "1"
    r = run_bass_kernel_spmd(nc, in_maps, core_ids=list(range(NCORES)),
                             trace=trace)
    LAST_RESULTS["r"] = r

    full = np.concatenate([r.results[c]["out"] for c in range(NCORES)], axis=0)
    return np.ascontiguousarray(full[:N_NODES]).astype(np.float32)


# revision 13
# speedup vs baseline: 1.7704x; 1.7704x over previous
"""GNN message-passing cell (3-step, 6 SpMMs) on 8 Trainium2 NeuronCores.

Strategy: 1D dest-node sharding. Each core owns 6272 rows (49 tiles of 128).
Per SpMM, edges are grouped by (dest core, source arrival-phase, dest tile);
neighbor features are fetched with dma_gather from an HBM-replicated state
table (built via chunked AllGather), scaled+segment-summed with a one-hot
matmul on the PE, accumulated in an SBUF accumulator, then LayerNorm+GELU.

v2: bf16 tables/streams/compute (PSUM accumulation stays f32), and
duplicate-spmm dedup: a (adjacency, source-state) pair appearing in several
steps is computed once and reused via an SBUF save buffer.
"""
import os
import sys

sys.path.insert(0, "/opt/trn_rl_repo")

import numpy as np

# ---------------- problem constants (hardcoded; must match reference) -------
N_STEP = 3
N_NODES = 50000
N_ADJ = 6
NNZ = 800000
DIN = 256
D = 128
LN_EPS = 1e-5

NCORES = 8
P = 128
TPC = 49                  # dest tiles per core
RPC = TPC * P             # 6272 rows per core
NPAD = NCORES * RPC       # 50176 padded rows
C0_T, C1_T = 25, 24       # shard-chunk split in tiles (for 2-chunk AllGather)
C0, C1 = C0_T * P, C1_T * P          # 3200 / 3072 rows per core per chunk
T0, T1 = NCORES * C0, NCORES * C1    # table chunk sizes: 25600 / 24576
WCH = 8                   # gather-window size in 128-edge chunks (1024 idxs:
                          # a single dma_gather >= 2048 idxs overflows the
                          # SWDGE descriptor carveout and wedges the device)
NSWQ = 4                  # SWDGE queues to rotate gathers over

DT_BF16 = os.environ.get("KERNEL_BF16", "1") == "1"
NEGPAD = os.environ.get("KERNEL_NEGPAD", "0") == "1"
DBG_STEPS = int(os.environ.get("KERNEL_STEPS", str(N_STEP)))  # debug bisection

LAST_RESULTS = {}         # test.py introspection (exec_time etc.)


# ---------------- host-side edge preprocessing ------------------------------
def _prep_spmm(rows, cols, vals):
    """Partition/sort/pad one adjacency's edges.

    Returns per-phase dicts with padded streams (tidx int16, dloc, val) laid
    out per core, plus the shared per-tile chunk counts K[p][t]. Padded slots
    get tidx=-1 (trailing within each tile group) so SWDGE skips their
    descriptors; dloc/val stay 0 so their one-hot column is zero.
    """
    rows = rows.astype(np.int64)
    cols = cols.astype(np.int64)
    dc = rows // RPC
    lr = rows % RPC
    t = lr // P
    dloc = lr % P
    cs = cols // RPC
    ls = cols % RPC
    ph = (ls >= C0).astype(np.int64)
    tidx = np.where(ph == 0, cs * C0 + ls, cs * C1 + (ls - C0))

    key = (dc * 2 + ph) * TPC + t
    # secondary sort by source index inside each group: gathered rows then
    # arrive mostly ascending -> HBM page locality for the random reads
    order = np.argsort(key * (1 << 16) + tidx, kind="stable")
    counts = np.bincount(key, minlength=NCORES * 2 * TPC).reshape(NCORES, 2, TPC)
    K = np.ceil(counts / P).astype(np.int64).max(axis=0)  # [2, TPC]
    K = np.maximum(K, 1)

    src_off = np.zeros(NCORES * 2 * TPC + 1, np.int64)
    np.cumsum(counts.reshape(-1), out=src_off[1:])
    # stream offsets per phase (tile groups packed in order)
    toff = [np.concatenate([[0], np.cumsum(K[p] * P)]) for p in range(2)]
    L = [int(toff[p][-1]) for p in range(2)]

    tidx_s = tidx[order]
    dloc_s = dloc[order]
    val_s = vals[order]

    pad_idx = -1 if NEGPAD else 0
    out = []
    for p in range(2):
        ti = np.full((NCORES, L[p]), pad_idx, np.int16)
        dl = np.zeros((NCORES, L[p]), np.float32)
        vl = np.zeros((NCORES, L[p]), np.float32)
        for c in range(NCORES):
            for tt in range(TPC):
                g = (c * 2 + p) * TPC + tt
                a, b = src_off[g], src_off[g + 1]
                n = b - a
                if n == 0:
                    continue
                o = toff[p][tt]
                ti[c, o:o + n] = tidx_s[a:b]
                dl[c, o:o + n] = dloc_s[a:b]
                vl[c, o:o + n] = val_s[a:b]
        out.append(dict(tidx=ti, dloc=dl, val=vl, L=L[p], K=K[p]))
    return out


def _groups(K):
    """Chunk-range [ga, gb) per dest tile from per-tile chunk counts."""
    g0 = np.concatenate([[0], np.cumsum(K)]).astype(np.int64)
    return [(t, int(g0[t]), int(g0[t + 1])) for t in range(TPC)]


def _wrap_idx(a):  # [L] int16 -> [128, L/16]
    return np.tile(a.reshape(-1, 16).T, (NCORES, 1)).astype(np.int16)


def _wrap_pe(a, np_dt):  # [L] -> [128, L/128] (edge e -> partition e%128)
    return np.ascontiguousarray(a.reshape(-1, P).T.astype(np_dt))


# ---------------- bass program ----------------------------------------------
def _build(meta):
    import concourse.bacc as bacc
    import concourse.mybir as mybir
    import concourse.tile as tile

    f32 = mybir.dt.float32
    i16 = mybir.dt.int16
    DT = mybir.dt.bfloat16 if DT_BF16 else f32
    Alu = mybir.AluOpType
    Act = mybir.ActivationFunctionType

    nc = bacc.Bacc("TRN2", target_bir_lowering=False, debug=False,
                   num_devices=NCORES, num_swdge_queues=NSWQ)

    xt_d = nc.dram_tensor("xt", [DIN, RPC], DT, kind="ExternalInput")
    w0_d = nc.dram_tensor("w0", [P, D], DT, kind="ExternalInput")
    w1_d = nc.dram_tensor("w1", [P, D], DT, kind="ExternalInput")
    brep_d = nc.dram_tensor("brep", [P, D], f32, kind="ExternalInput")
    grep_d = nc.dram_tensor("grep", [P, D], f32, kind="ExternalInput")
    berep_d = nc.dram_tensor("berep", [P, D], f32, kind="ExternalInput")
    iota_d = nc.dram_tensor("iotar", [P, D], DT, kind="ExternalInput")
    idx_d, dloc_d, val_d = {}, {}, {}
    for (m, p), L in meta["lengths"].items():
        idx_d[(m, p)] = nc.dram_tensor(f"idx_{m}_{p}", [P, L // 16], i16,
                                       kind="ExternalInput")
        dloc_d[(m, p)] = nc.dram_tensor(f"dloc_{m}_{p}", [P, L // P], DT,
                                        kind="ExternalInput")
        val_d[(m, p)] = nc.dram_tensor(f"val_{m}_{p}", [P, L // P], DT,
                                       kind="ExternalInput")
    out_d = nc.dram_tensor("out", [RPC, D], f32, kind="ExternalOutput")

    ts = lambda t: slice(t * D, (t + 1) * D)
    step_items = meta["steps"][:DBG_STEPS]
    dup_sources = meta["dup_sources"]
    KMAX = meta["kmax"]

    with tile.TileContext(nc) as tc:
        with (
            tc.tile_pool(name="const", bufs=1) as cp,
            tc.tile_pool(name="acc", bufs=1) as ap_,
            tc.tile_pool(name="xp", bufs=3) as xp,
            tc.tile_pool(name="gp", bufs=8) as gp,
            tc.tile_pool(name="op", bufs=8) as op_,
            tc.tile_pool(name="ip", bufs=2) as ip,
            tc.tile_pool(name="sp", bufs=4) as stp,
            tc.tile_pool(name="ps", bufs=8, space="PSUM") as pp,
            tc.tile_pool(name="dr", bufs=1, space="DRAM") as dp,
        ):
            w0_t = cp.tile([P, D], DT, name="w0t")
            nc.sync.dma_start(w0_t[:], w0_d[:])
            w1_t = cp.tile([P, D], DT, name="w1t")
            nc.sync.dma_start(w1_t[:], w1_d[:])
            brep_t = cp.tile([P, D], f32, name="brept")
            nc.sync.dma_start(brep_t[:], brep_d[:])
            grep_t = cp.tile([P, D], f32, name="grept")
            nc.sync.dma_start(grep_t[:], grep_d[:])
            berep_t = cp.tile([P, D], f32, name="berept")
            nc.sync.dma_start(berep_t[:], berep_d[:])
            iota_t = cp.tile([P, D], DT, name="iotat")
            nc.sync.dma_start(iota_t[:], iota_d[:])
            eps_t = cp.tile([P, 1], f32, name="epst")
            nc.vector.memset(eps_t[:], LN_EPS)

            accum = ap_.tile([P, TPC * D], f32, name="accum")
            saves = {sid: ap_.tile([P, TPC * D], f32, name=f"save{sid}")
                     for sid in dup_sources}

            # memset gather tiles once: SWDGE skips negative (padded) indices,
            # leaving stale SBUF bytes that feed the PE via a zero one-hot
            # column -- they must be finite (0 * Inf/NaN would poison PSUM).
            if NEGPAD:
                # SWDGE skips negative (padded) indices, leaving stale SBUF
                # bytes that feed the PE via a zero one-hot column -- they
                # must be finite (0 * Inf/NaN would poison PSUM).
                for _ in range(8):
                    gz = gp.tile([P, WCH, D], DT, tag="g")
                    nc.vector.memset(gz[:], 0.0)

            tables = {}

            def exchange_chunk(s, ch):
                eng = nc.gpsimd if DT_BF16 else nc.sync
                if ch == 0:
                    agi0 = dp.tile([C0, D], DT, name=f"agi0_{s}", tag=f"agi0_{s}")
                    tab0 = dp.tile([T0, D], DT, name=f"tab0_{s}", tag=f"tab0_{s}",
                                   addr_space="Shared")
                    eng.dma_start(
                        agi0[:].rearrange("(t p) f -> p t f", p=P),
                        accum[:, :C0_T * D].rearrange("p (t f) -> p t f", f=D))
                    nc.gpsimd.collective_compute(
                        "AllGather", Alu.bypass,
                        replica_groups=[list(range(NCORES))],
                        ins=[agi0[:]], outs=[tab0[:]])
                    tables.setdefault(s, [None, None])[0] = tab0
                else:
                    agi1 = dp.tile([C1, D], DT, name=f"agi1_{s}", tag=f"agi1_{s}")
                    tab1 = dp.tile([T1, D], DT, name=f"tab1_{s}", tag=f"tab1_{s}",
                                   addr_space="Shared")
                    eng.dma_start(
                        agi1[:].rearrange("(t p) f -> p t f", p=P),
                        accum[:, C0_T * D:].rearrange("p (t f) -> p t f", f=D))
                    nc.gpsimd.collective_compute(
                        "AllGather", Alu.bypass,
                        replica_groups=[list(range(NCORES))],
                        ins=[agi1[:]], outs=[tab1[:]])
                    tables.setdefault(s, [None, None])[1] = tab1

            # ---------------- LayerNorm + GELU (fused per-tile) -------------
            def ln_tile(t, src):
                X = src[:, ts(t)]
                s1 = stp.tile([P, 1], f32, tag="s1")
                nc.vector.reduce_sum(out=s1[:], in_=X, axis=mybir.AxisListType.X)
                mean = stp.tile([P, 1], f32, tag="mean")
                nc.vector.tensor_scalar_mul(out=mean[:], in0=s1[:],
                                            scalar1=1.0 / D)
                xc = xp.tile([P, D], f32, tag="xc")
                nc.vector.tensor_scalar(out=xc[:], in0=X, scalar1=mean[:],
                                        scalar2=None, op0=Alu.subtract)
                sq = xp.tile([P, D], f32, tag="sq")
                nc.vector.tensor_tensor(out=sq[:], in0=xc[:], in1=xc[:],
                                        op=Alu.mult)
                v1 = stp.tile([P, 1], f32, tag="v1")
                nc.vector.reduce_sum(out=v1[:], in_=sq[:], axis=mybir.AxisListType.X)
                sd = stp.tile([P, 1], f32, tag="sd")
                nc.scalar.activation(out=sd[:], in_=v1[:], func=Act.Sqrt,
                                     bias=eps_t[:], scale=1.0 / D)
                rstd = stp.tile([P, 1], f32, tag="rstd")
                nc.vector.reciprocal(out=rstd[:], in_=sd[:])
                y = xp.tile([P, D], f32, tag="y")
                nc.vector.tensor_scalar(out=y[:], in0=xc[:], scalar1=rstd[:],
                                        scalar2=None, op0=Alu.mult)
                nc.vector.tensor_tensor(out=y[:], in0=y[:], in1=grep_t[:],
                                        op=Alu.mult)
                nc.vector.tensor_tensor(out=y[:], in0=y[:], in1=berep_t[:],
                                        op=Alu.add)
                yo = xp.tile([P, D], f32, tag="yo")
                nc.scalar.activation(out=yo[:], in_=y[:], func=Act.Gelu)
                nc.sync.dma_start(out_d[t * P:(t + 1) * P, :], yo[:])

            # ---------------- affine: h0 = x @ W + b (interleaved with ------
            # ---------------- the first AllGather's two chunks) -------------
            for t in range(TPC):
                xt0 = xp.tile([P, P], DT, tag="xt0")
                nc.sync.dma_start(xt0[:], xt_d[0:P, t * P:(t + 1) * P])
                xt1 = xp.tile([P, P], DT, tag="xt1")
                nc.sync.dma_start(xt1[:], xt_d[P:DIN, t * P:(t + 1) * P])
                ps = pp.tile([P, D], mybir.dt.float32, name="psa", tag="ps")
                nc.tensor.matmul(out=ps[:], lhsT=xt0[:], rhs=w0_t[:],
                                 start=True, stop=False)
                nc.tensor.matmul(out=ps[:], lhsT=xt1[:], rhs=w1_t[:],
                                 start=False, stop=True)
                nc.vector.tensor_tensor(out=accum[:, ts(t)], in0=ps[:],
                                        in1=brep_t[:], op=Alu.add)
            exchange_chunk(0, 0)
            exchange_chunk(0, 1)

            # ---------------- message-passing steps ----------------

            for i, items in enumerate(step_items):
                first = [True] * TPC
                for j, it in enumerate(items):
                    if it[0] == "reuse":
                        sv = saves[it[1]]
                        for t in range(TPC):
                            if first[t]:
                                nc.vector.tensor_copy(out=accum[:, ts(t)],
                                                      in_=sv[:, ts(t)])
                                first[t] = False
                            else:
                                nc.vector.tensor_tensor(
                                    out=accum[:, ts(t)], in0=accum[:, ts(t)],
                                    in1=sv[:, ts(t)], op=Alu.add)
                        continue
                    _, m, sigma = it
                    is_src = m in dup_sources
                    first_sv = [True] * TPC
                    for p in range(2):
                        L = meta["lengths"][(m, p)]
                        idx_t = ip.tile([P, L // 16], i16, tag="idx")
                        nc.sync.dma_start(idx_t[:], idx_d[(m, p)][:])
                        dloc_t = ip.tile([P, L // P], DT, tag="dloc")
                        nc.sync.dma_start(dloc_t[:], dloc_d[(m, p)][:])
                        val_t = ip.tile([P, L // P], DT, tag="val")
                        nc.sync.dma_start(val_t[:], val_d[(m, p)][:])
                        tab = tables[sigma][p]
                        groups = meta["groups"][(m, p)]
                        total_ch = L // P
                        nwin = (total_ch + WCH - 1) // WCH
                        gi = 0          # group cursor
                        ps = None
                        for w in range(nwin):
                            w0c = w * WCH
                            wlen = min(WCH, total_ch - w0c)
                            nidx = wlen * P
                            g_t = gp.tile([P, WCH, D], DT, tag="g")
                            nc.gpsimd.dma_gather(
                                g_t[:, :wlen, :], tab[:],
                                idx_t[:, w0c * 8:(w0c + wlen) * 8],
                                nidx, nidx, D,
                                queue_num=w % NSWQ)
                            oh = op_.tile([P, WCH, D], DT, tag="oh")
                            dloc_b = dloc_t[:, w0c:w0c + wlen].rearrange(
                                "p (c o) -> p c o", o=1).to_broadcast(
                                [P, wlen, D])
                            val_b = val_t[:, w0c:w0c + wlen].rearrange(
                                "p (c o) -> p c o", o=1).to_broadcast(
                                [P, wlen, D])
                            iota_b = iota_t[:].rearrange(
                                "p (c b) -> p c b", c=1).to_broadcast(
                                [P, wlen, D])
                            nc.vector.tensor_tensor(
                                out=oh[:, :wlen, :], in0=dloc_b, in1=iota_b,
                                op=Alu.is_equal)
                            nc.vector.tensor_tensor(
                                out=oh[:, :wlen, :], in0=oh[:, :wlen, :],
                                in1=val_b, op=Alu.mult)
                            for kk in range(w0c, w0c + wlen):
                                t, ga, gb = groups[gi]
                                if kk == ga:
                                    ps = pp.tile([P, D], mybir.dt.float32,
                                                 name="psm", tag="ps")
                                nc.tensor.matmul(
                                    out=ps[:], lhsT=oh[:, kk - w0c, :],
                                    rhs=g_t[:, kk - w0c, :],
                                    start=(kk == ga), stop=(kk == gb - 1))
                                if kk == gb - 1:
                                    if first[t]:
                                        nc.vector.tensor_copy(
                                            out=accum[:, ts(t)], in_=ps[:])
                                        first[t] = False
                                    else:
                                        nc.vector.tensor_tensor(
                                            out=accum[:, ts(t)],
                                            in0=accum[:, ts(t)], in1=ps[:],
                                            op=Alu.add)
                                    if is_src:
                                        if first_sv[t]:
                                            nc.vector.tensor_copy(
                                                out=saves[m][:, ts(t)],
                                                in_=ps[:])
                                            first_sv[t] = False
                                        else:
                                            nc.vector.tensor_tensor(
                                                out=saves[m][:, ts(t)],
                                                in0=saves[m][:, ts(t)],
                                                in1=ps[:], op=Alu.add)
                                    gi += 1
                if i < len(step_items) - 1:
                    exchange_chunk(i + 1, 0)
                    exchange_chunk(i + 1, 1)

            for t in range(TPC):
                ln_tile(t, accum)

    nc.compile()
    n_inst = sum(len(b.instructions) for f in nc.m.functions for b in f.blocks)
    print(f"[kernel] instructions: {n_inst}", flush=True)
    return nc


# ---------------- entry point ------------------------------------------------
def kernel(x, adj_rows, adj_cols, adj_vals, idxes_seq, idxes_res, W, b,
           gamma, beta):
    from concourse.bass_utils import run_bass_kernel_spmd

    import ml_dtypes
    np_DT = ml_dtypes.bfloat16 if DT_BF16 else np.float32

    x = np.asarray(x, np.float32)
    W = np.asarray(W, np.float32)
    b = np.asarray(b, np.float32)
    gamma = np.asarray(gamma, np.float32)
    beta = np.asarray(beta, np.float32)
    adj_rows = np.asarray(adj_rows)
    adj_cols = np.asarray(adj_cols)
    adj_vals = np.asarray(adj_vals, np.float32)
    idxes_seq = np.asarray(idxes_seq).astype(np.int64)
    idxes_res = np.asarray(idxes_res).astype(np.int64)

    # spmm list per step with residuals first (their tables exist already, so
    # their gathers overlap the in-flight AllGather of the fresh state).
    # Duplicate (adjacency, state) pairs are computed once and reused via an
    # SBUF save buffer.
    stream_of = {}           # (a, sigma) -> stream id
    stream_specs = []        # sid -> (a, sigma)
    step_items = []          # per step: ("spmm", sid, sigma) | ("reuse", sid)
    dup_sources = set()
    off = 0
    for i in range(N_STEP):
        raw = []
        for j in range(i):
            raw.append((int(idxes_res[off + j]), j))
        raw.append((int(idxes_seq[i]), i))
        off += i
        items = []
        for key in raw:
            if key in stream_of:
                dup_sources.add(stream_of[key])
                items.append(("reuse", stream_of[key]))
            else:
                sid = len(stream_specs)
                stream_of[key] = sid
                stream_specs.append(key)
                items.append(("spmm", sid, key[1]))
        step_items.append(items)

    # host prep per unique stream
    lengths, groups = {}, {}
    per_core_streams = {}     # (sid,p) -> dict arrays per core
    kmax = 1
    for sid, (a, sigma) in enumerate(stream_specs):
        phases = _prep_spmm(adj_rows[a], adj_cols[a], adj_vals[a])
        for p in range(2):
            ph = phases[p]
            lengths[(sid, p)] = ph["L"]
            groups[(sid, p)] = _groups(ph["K"])
            kmax = max(kmax, int(ph["K"].max()))
            per_core_streams[(sid, p)] = ph

    meta = dict(lengths=lengths, groups=groups, steps=step_items,
                dup_sources=dup_sources, kmax=kmax)
    nc = _build(meta)

    # per-core inputs
    xpad = np.zeros((NPAD, DIN), np.float32)
    xpad[:N_NODES] = x
    xt_full = np.ascontiguousarray(xpad.T)

    iota_rep = np.tile(np.arange(D, dtype=np.float32), (P, 1))
    in_maps = []
    for c in range(NCORES):
        im = dict(
            xt=np.ascontiguousarray(
                xt_full[:, c * RPC:(c + 1) * RPC]).astype(np_DT),
            w0=W[:P].astype(np_DT),
            w1=W[P:].astype(np_DT),
            brep=np.tile(b, (P, 1)).astype(np.float32),
            grep=np.tile(gamma, (P, 1)).astype(np.float32),
            berep=np.tile(beta, (P, 1)).astype(np.float32),
            iotar=iota_rep.astype(np_DT),
        )
        for (m, p), ph in per_core_streams.items():
            im[f"idx_{m}_{p}"] = _wrap_idx(ph["tidx"][c])
            im[f"dloc_{m}_{p}"] = _wrap_pe(ph["dloc"][c], np_DT)
            im[f"val_{m}_{p}"] = _wrap_pe(ph["val"][c], np_DT)
        in_maps.append(im)

    trace = os.environ.get("KERNEL_TRACE", "0") == "1"
    r = run_bass_kernel_spmd(nc, in_maps, core_ids=list(range(NCORES)),
                             trace=trace)
    LAST_RESULTS["r"] = r

    full = np.concatenate([r.results[c]["out"] for c in range(NCORES)], axis=0)
    return np.ascontiguousarray(full[:N_NODES]).astype(np.float32)
